# revision 1
# baseline (speedup 1.0000x reference)
"""Trainium2 Bass kernel for nn_AsynBaseStem (sparse 7x7 conv + BN + ReLU +
scatter + 3x3/2 maxpool), 8-core data-parallel over output row bands.

Architecture (per core, fully dense, no indirect DMA):
  - Host prebuilds a [128, 81*646] bf16 operand table T6 per core:
      rows 0..125  : (j,i,ch) j<6 -> fm_pad[r+i, c+j, ch]  (column-shifted planar stripes)
      row  126     : inactive flag (1.0 where pixel has no site, else 0.0)
      row  127     : ones (bias row)
  - Dense conv at every pixel via 2 accumulating matmuls (K=128 main + K=21
    tail read from T6 rows 0..20 at col offset +6). The flag row adds -1e9 to
    inactive pixels (masking), the ones row adds the BN bias.
  - PSUM eviction fuses the column max-pool (DVE even/odd max + ACT third-col
    copy), then a row ring-buffer completes the 3x3/2 max pool.
  - Final ReLU folded into the row pool; one cast-DMA writes [64, p*320] f32;
    the host transposes to [p, q, ch] during unsharding.

kernel(**inputs) takes FULL unsharded inputs, returns [319, 319, 64] f32.
"""
import numpy as np
import ml_dtypes
from contextlib import ExitStack

H = W = 640
CIN, COUT = 3, 64
K, PAD = 7, 3
NCORES = 8
BROWS = 81            # dense rows per core band
WPAD = W + 2 * PAD    # 646
NB = BROWS * WPAD     # T6 free size per core
NBP = NB + 8          # +pad so the tail matmul window (x+6) stays in bounds
PROWS = 40            # pooled rows per core (core 7: 39 valid)
QCOLS = 319
BN_EPS = 1e-5
NEG = -1.0e9


def _build_bass():
    import concourse.bass as bass
    import concourse.mybir as mybir
    import concourse.tile as tile
    from concourse import bacc

    fp32 = mybir.dt.float32
    bf16 = mybir.dt.bfloat16

    nc = bacc.Bacc()
    t6_ext = nc.declare_dram_parameter("t6", [128, NBP], bf16, isOutput=False)
    # packed params: [w | wtail(pad128) | sel126 | sel127 | gam | bet | mu | var]
    par_ext = nc.declare_dram_parameter("par", [128, 8 * COUT], fp32, isOutput=False)
    out_ext = nc.declare_dram_parameter("out", [COUT, PROWS * 320], fp32, isOutput=True)

    with ExitStack() as ctx:
        tc = ctx.enter_context(tile.TileContext(nc))
        cpool = ctx.enter_context(tc.tile_pool(name="const", bufs=1))
        rowp = ctx.enter_context(tc.tile_pool(name="rowbufs", bufs=12))
        ringp = ctx.enter_context(tc.tile_pool(name="ring", bufs=1))
        psp = ctx.enter_context(tc.tile_pool(name="ps", bufs=8, space="PSUM"))

        # ---- weight prep: lhsT A [128, 64] (W'[0:126] + flag row + bias row),
        #      lhsT B [21, 64] (W'[126:147]); W' = W * inv, inv = gamma*rsqrt(var+eps)
        par = cpool.tile([128, 8 * COUT], fp32)
        nc.sync.dma_start(par[:], par_ext[:])
        C = COUT
        wa_f = par[:, 0:C]
        wb_f = par[0:21, C:C + C]
        s126 = par[:, 2 * C:3 * C]
        s127 = par[:, 3 * C:4 * C]
        gam = par[:, 4 * C:5 * C]
        bet = par[:, 5 * C:6 * C]
        mu = par[:, 6 * C:7 * C]
        var = par[:, 7 * C:8 * C]

        # ---- big operand table: chunked load AFTER the small parameter DMAs
        # (HWDGE is FIFO per queue) so weight prep and the first conv rows
        # don't wait for the full 13.4MB stream
        t6 = cpool.tile([128, NBP], bf16)
        bounds = [0, 2] + [2 + 10 * i for i in range(1, 8)] + [BROWS]
        for ck in range(len(bounds) - 1):
            sl = slice(bounds[ck] * WPAD,
                       bounds[ck + 1] * WPAD if ck + 2 < len(bounds) else NBP)
            nc.sync.dma_start(t6[:, sl], t6_ext[:, sl])

        # lhsA = wa*inv + selD*bias' + selN, where selD = sel127 - sel126 and
        # selN = -1e9 at row 126 (host constants); bias' = bet - mu*inv
        inv = cpool.tile([128, COUT], fp32)
        nc.vector.tensor_scalar_add(inv[:], var, BN_EPS)
        nc.scalar.activation(inv[:], inv[:], mybir.ActivationFunctionType.Sqrt)
        nc.vector.reciprocal(inv[:], inv[:])
        nc.vector.tensor_mul(inv[:], inv[:], gam)
        u = cpool.tile([128, COUT], fp32)
        nc.vector.tensor_mul(u[:], mu, inv[:])
        nc.vector.tensor_sub(u[:], bet, u[:])          # u = bias'
        nc.vector.tensor_mul(u[:], u[:], s126)         # u = selD*bias'
        acc = cpool.tile([128, COUT], fp32)
        nc.vector.tensor_mul(acc[:], wa_f, inv[:])
        nc.vector.tensor_add(acc[:], acc[:], s127)     # + selN
        lhsA = cpool.tile([128, COUT], bf16)
        nc.vector.tensor_add(lhsA[:], acc[:], u[:])
        lhsB = cpool.tile([21, COUT], bf16)
        nc.vector.tensor_mul(lhsB[:], wb_f, inv[0:21, :])

        # ---- pooled accumulator [64, PROWS, 320] bf16 and row ring ----
        pooled = ringp.tile([COUT, PROWS * 320], bf16)
        mring = ringp.tile([COUT, 8 * 320], bf16)  # m rows modulo 8

        # Continuous-pixel-space conv: N=512 matmul tiles over x in [0, NB).
        # Row-boundary/pad pixels carry flag=1 -> -1e9, so the pool ignores
        # them. Per-row ev (even cols) and t (pair-max) staging buffers absorb
        # tile fragments; a full-row m then feeds the row pool.
        NT = (NB + 511) // 512
        evrow = {}
        trow = {}

        def finish_row(r):
            mrow = mring[:, (r % 8) * 320:(r % 8) * 320 + 320]
            nc.vector.tensor_tensor(
                out=mrow[:], in0=trow[r][:, 0:320], in1=evrow[r][:, 1:321],
                op=mybir.AluOpType.max)
            del evrow[r], trow[r]
            if r >= 2 and r % 2 == 0:
                p = (r - 2) // 2
                m0 = mring[:, ((r - 2) % 8) * 320:((r - 2) % 8) * 320 + 320]
                m1 = mring[:, ((r - 1) % 8) * 320:((r - 1) % 8) * 320 + 320]
                s01 = rowp.tile([COUT, 320], bf16, tag="s01")
                nc.vector.tensor_tensor(out=s01[:], in0=m0[:], in1=m1[:],
                                        op=mybir.AluOpType.max)
                po = pooled[:, p * 320:(p + 1) * 320]
                nc.vector.scalar_tensor_tensor(
                    out=po[:], in0=s01[:], scalar=0.0, in1=mrow[:],
                    op0=mybir.AluOpType.max, op1=mybir.AluOpType.max)
                # stream pooled rows out in chunks of 10 (cast bf16 -> f32);
                # host does the final [p,q,ch] transpose during unsharding
                if p % 5 == 4:
                    pc = p // 5
                    nc.gpsimd.dma_start(
                        out_ext[:, pc * 1600:(pc + 1) * 1600],
                        pooled[:, pc * 1600:(pc + 1) * 1600])

        for k in range(NT):
            xa = 512 * k
            xb = min(xa + 512, NB)
            wdt = xb - xa
            ps = psp.tile([COUT, 512], fp32, tag="convps")
            nc.tensor.matmul(ps[:, 0:wdt], lhsA[:], t6[0:128, xa:xb],
                             start=True, stop=False)
            nc.tensor.matmul(ps[:, 0:wdt], lhsB[:], t6[0:21, xa + 6:xb + 6],
                             start=False, stop=True)
            for r in range(xa // WPAD, (xb - 1) // WPAD + 1):
                ca = max(xa, r * WPAD) - r * WPAD     # even
                cb = min(xb, r * WPAD + WPAD) - r * WPAD  # even
                if r not in evrow:
                    evrow[r] = rowp.tile([COUT, 324], bf16, tag="evrow", name=f"evrow{r}")
                    trow[r] = rowp.tile([COUT, 324], bf16, tag="trowb", name=f"trowb{r}")
                ne = (cb - ca) // 2
                p0 = r * WPAD + ca - xa               # psum-local offset
                nc.scalar.copy(evrow[r][:, ca // 2:ca // 2 + ne],
                               ps[:, p0:p0 + 2 * ne:2])
                nc.vector.tensor_tensor(
                    out=trow[r][:, ca // 2:ca // 2 + ne],
                    in0=evrow[r][:, ca // 2:ca // 2 + ne],
                    in1=ps[:, p0 + 1:p0 + 2 * ne:2],
                    op=mybir.AluOpType.max)
                if cb == WPAD:
                    finish_row(r)


    nc.finalize()
    return nc


_NC_CACHE = None


def _get_nc():
    global _NC_CACHE
    if _NC_CACHE is None:
        _NC_CACHE = _build_bass()
    return _NC_CACHE


def build_in_maps(update_location, feature_map, weight, gamma, beta,
                  running_mean, running_var):
    fm = np.asarray(feature_map, np.float32)
    loc = np.asarray(update_location).astype(np.int64)
    wt = np.asarray(weight, np.float32)

    fm_pad = np.pad(fm, ((PAD, PAD), (PAD, PAD), (0, 0)))          # [646,646,3]
    # stripes B_T[t=(i,ch), r, c] = fm_pad[r+i, c, ch], r in 0..640 (row 640 pad)
    bt = np.zeros((21, H + 1, WPAD), np.float32)
    for i in range(K):
        for ch in range(CIN):
            bt[i * CIN + ch, 0:H, :] = fm_pad[i:i + H, :, ch]
    bt = bt.astype(ml_dtypes.bfloat16)

    # inactive flag = 1 where no site; indexed by output pixel (r, c) at
    # position c in the 646-pitch row; columns 640..645 stay inactive.
    flag = np.ones((H + 1, WPAD), np.float32)
    flag[loc[:, 0], loc[:, 1]] = 0.0
    flag[:, H:] = 1.0
    flag = flag.astype(ml_dtypes.bfloat16)

    # reordered weights W_re[(j,i,ch), o] = weight[i, j, ch, o]
    w_re = np.ascontiguousarray(
        wt.transpose(1, 0, 2, 3).reshape(147, COUT)).astype(np.float32)

    bcast = lambda v: np.ascontiguousarray(
        np.broadcast_to(np.asarray(v, np.float32)[None, :], (128, COUT)))

    in_maps = []
    for k in range(NCORES):
        r0 = 80 * k
        t6 = np.zeros((128, BROWS, WPAD), ml_dtypes.bfloat16)
        for j in range(6):
            sl = bt[:, r0:r0 + BROWS, :]
            t6[j * 21:(j + 1) * 21, :, :-j or None] = sl[:, :, j:]
        t6[126] = flag[r0:r0 + BROWS]
        t6[127] = np.ones((BROWS, WPAD), ml_dtypes.bfloat16)
        wfull = np.zeros((128, COUT), np.float32)
        wfull[0:126] = w_re[0:126]
        sel126 = np.zeros((128, COUT), np.float32)   # selD: +1 at row 127, -1 at 126
        sel126[127] = 1.0
        sel126[126] = -1.0
        sel127 = np.zeros((128, COUT), np.float32)   # selN: NEG at row 126
        sel127[126] = NEG
        t6p = np.zeros((128, NBP), ml_dtypes.bfloat16)
        t6p[:, :NB] = t6.reshape(128, NB)
        wtail = np.zeros((128, COUT), np.float32)
        wtail[0:21] = w_re[126:147]
        par = np.concatenate([wfull, wtail, sel126, sel127, bcast(gamma),
                              bcast(beta), bcast(running_mean),
                              bcast(running_var)], axis=1)
        in_maps.append({"t6": t6p, "par": np.ascontiguousarray(par)})
    return in_maps


def kernel(update_location, feature_map, weight, gamma, beta, running_mean,
           running_var):
    from concourse.bass_utils import run_bass_kernel_spmd

    in_maps = build_in_maps(update_location, feature_map, weight, gamma, beta,
                            running_mean, running_var)
    nc = _get_nc()
    res = run_bass_kernel_spmd(nc, in_maps, core_ids=list(range(NCORES)))
    # per-core out is [64, PROWS*320] f32 (ch-major); assemble [319, 319, 64]
    parts = []
    for k in range(NCORES):
        o = res.results[k]["out"].reshape(COUT, PROWS, 320)
        parts.append(o.transpose(1, 2, 0)[:, :QCOLS, :])
    out = np.concatenate(parts, axis=0)[:QCOLS]
    return np.ascontiguousarray(out).astype(np.float32)



# revision 2
# speedup vs baseline: 1.1414x; 1.1414x over previous
"""Trainium2 Bass kernel v2 for nn_AsynBaseStem: dense masked 7x7 conv + BN +
ReLU + 3x3/2 maxpool, 8-core data-parallel over output row bands.

Per-core architecture (vs v1 baseline):
  - 4 concurrent PE streams via 64x64 tile_position quadrants: SBUF partition
    halves hold two 64-row operand tables (group0 = dense rows 0..40, group1 =
    rows 40..80 of the core's 80-row band); each group feeds two quadrant
    streams covering two 21-row sub-bands (A/B and C/D).
  - Table rows = 63 column-shift stripes (j' in 0..2, i in 0..6, ch in 0..2)
    + 1 inactive-flag row; conv = 3 accumulating K=64 passes reading the same
    table at column offsets 0/+3/+6 (j = j'+3p), BN scale folded into weights
    on the host. Flag row weight -1e9 in pass0 masks inactive pixels.
  - PSUM tiles are [128, 1024] fp32 (2 banks): lower 64 partitions = sub-band
    A (or C) pixels, upper = B (or D) at the same band-local x; row pitch 648
    keeps both halves row-aligned so all pool ops run on 128 partitions.
  - Eviction per tile: ACT copies shifted evens (evs[v]=c[2v+2], bf16), DVE
    pair-max t[v]=max(c[2v],c[2v+1]) via evs[v-1] + PSUM odds; per-row
    m = max(t, evs) runs in DVE 2x bf16 mode; vertical 3-row max, final ACT
    ReLU(x+bias'), bf16 DMA out. Host does BN folding and the fp32 cast.
"""
import numpy as np
import ml_dtypes
from contextlib import ExitStack

H = W = 640
CIN, COUT = 3, 64
KK, PAD = 7, 3
NCORES = 8
NEG = -1.0e9
BN_EPS = 1e-5

WP = 648                  # row pitch (even half-pitch 324 keeps m-max aligned)
GROWS = 41                # dense rows per partition-group table
GCOLS = GROWS * WP        # 26568
GPAD = GCOLS + 8          # +8 so +6-offset reads stay in bounds
BROWS = 21                # dense rows per sub-band
TSPAN = 1022              # full tile advance (1024 cols computed, 2 overlap)
NFULL = 13                # full tiles per sub-band
LSPAN = 324               # last partial tile: x 13286..13610
NPOOL = 10                # pooled rows per sub-band
MPITCH = 324
VMAX = BROWS * MPITCH     # 6804 = evs/t/m buffer cols
BX0 = 20 * WP             # upper sub-band band-local x origin (12960)


def _build_bass():
    import concourse.bass as bass
    import concourse.mybir as mybir
    import concourse.tile as tile
    from concourse import bacc

    fp32 = mybir.dt.float32
    bf16 = mybir.dt.bfloat16
    MAX = mybir.AluOpType.max

    nc = bacc.Bacc()
    tbl_ext = nc.declare_dram_parameter("tbl", [128, GPAD], bf16, isOutput=False)
    wt_ext = nc.declare_dram_parameter("wt", [128, 192], bf16, isOutput=False)
    bias_ext = nc.declare_dram_parameter("bias", [128, 1], fp32, isOutput=False)
    out_ext = nc.declare_dram_parameter("out", [128, 2 * NPOOL * 320], bf16,
                                        isOutput=True)

    with ExitStack() as ctx:
        tc = ctx.enter_context(tile.TileContext(nc))
        cpool = ctx.enter_context(tc.tile_pool(name="const", bufs=1))
        psp = ctx.enter_context(tc.tile_pool(name="ps", bufs=2, space="PSUM"))

        wt = cpool.tile([128, 192], bf16)
        nc.sync.dma_start(wt[:], wt_ext[:])
        bias = cpool.tile([128, 1], fp32)
        nc.sync.dma_start(bias[:], bias_ext[:])

        # table: chunk loads ordered so all 4 sub-band streams start early;
        # triggers spread across idle engine queues to parallelize issue
        tbl = cpool.tile([128, GPAD], bf16)
        chunks = [(0, 1), (20, 21), (1, 3), (21, 23), (3, 9), (23, 29),
                  (9, 16), (29, 36), (16, 20), (36, 41)]
        for (r0, r1) in chunks:
            a = r0 * WP
            b = r1 * WP if r1 < GROWS else GPAD
            nc.sync.dma_start(tbl[:, a:b], tbl_ext[:, a:b])

        ev0 = cpool.tile([128, 2], bf16)
        evs = [cpool.tile([128, VMAX], bf16, name=f"evs{c}") for c in range(2)]
        tb = [cpool.tile([128, VMAX], bf16, name=f"tb{c}") for c in range(2)]
        mb = [cpool.tile([128, VMAX], bf16, name=f"mb{c}") for c in range(2)]
        s01 = [cpool.tile([128, NPOOL * 320], bf16, name=f"s01_{c}")
               for c in range(2)]
        pooled = [cpool.tile([128, NPOOL * 320], bf16, name=f"pool{c}")
                  for c in range(2)]
        evs3 = [a.rearrange("p (a b) -> p a b", b=MPITCH) for a in evs]
        tb3 = [a.rearrange("p (a b) -> p a b", b=MPITCH) for a in tb]
        mb3 = [a.rearrange("p (a b) -> p a b", b=MPITCH) for a in mb]
        s013 = [a.rearrange("p (a b) -> p a b", b=320) for a in s01]
        pooled3 = [a.rearrange("p (a b) -> p a b", b=320) for a in pooled]

        mdone = [0, 0]   # m rows emitted per class
        pdone = [0, 0]   # pooled rows emitted per class
        odone = [0, 0]   # pooled rows relu'd + DMA'd per class

        def emit_m(cls, upto):
            r0 = mdone[cls]
            while r0 < upto:
                n = min(5, upto - r0)
                nc.vector.tensor_tensor(
                    out=mb3[cls][:, r0:r0 + n, 0:321],
                    in0=tb3[cls][:, r0:r0 + n, 0:321],
                    in1=evs3[cls][:, r0:r0 + n, 0:321],
                    op=MAX)
                r0 += n
            mdone[cls] = upto

        def emit_pool(cls, upto):
            p0 = pdone[cls]
            while p0 < upto:
                n = min(5, upto - p0)
                nc.vector.tensor_tensor(
                    out=s013[cls][:, p0:p0 + n, :],
                    in0=mb3[cls][:, 2 * p0:2 * p0 + 2 * n:2, 0:320],
                    in1=mb3[cls][:, 2 * p0 + 1:2 * p0 + 2 * n:2, 0:320],
                    op=MAX)
                nc.vector.tensor_tensor(
                    out=pooled3[cls][:, p0:p0 + n, :],
                    in0=s013[cls][:, p0:p0 + n, :],
                    in1=mb3[cls][:, 2 * p0 + 2:2 * p0 + 2 * n + 1:2, 0:320],
                    op=MAX)
                p0 += n
            pdone[cls] = upto

        def emit_out(cls, upto):
            a = odone[cls] * 320
            b = upto * 320
            nc.scalar.activation(pooled[cls][:, a:b], pooled[cls][:, a:b],
                                 mybir.ActivationFunctionType.Relu,
                                 bias=bias[:, 0:1])
            nc.gpsimd.dma_start(
                out_ext[:, cls * NPOOL * 320 + a:cls * NPOOL * 320 + b],
                pooled[cls][:, a:b])
            odone[cls] = upto

        for Wt in range(NFULL + 1):
            full = Wt < NFULL
            ncols = 1024 if full else LSPAN
            x0 = Wt * TSPAN
            v0 = x0 // 2            # = 511*Wt
            # --- conv: both classes interleaved so all 4 quadrants run ---
            nsw = 2 if full else 1
            pst = [psp.tile([128, 1024], fp32, tag=f"ps{c}", name=f"ps{c}_{Wt}")
                   for c in range(2)]
            for p in range(3):
                st, sp = (p == 0), (p == 2)
                for half in range(2):
                    hx0 = x0 + (BX0 if half else 0)
                    ob = 64 * half
                    for sw in range(nsw):
                        nw = min(512, ncols - 512 * sw)
                        for cls in range(2):
                            gp = cls * 64
                            nc.tensor.matmul(
                                pst[cls][ob:ob + 64, 512 * sw:512 * sw + nw],
                                wt[gp:gp + 64, 64 * p:64 * p + 64],
                                tbl[gp:gp + 64,
                                    hx0 + 512 * sw + 3 * p:
                                    hx0 + 512 * sw + 3 * p + nw],
                                start=st, stop=sp)
            for cls in range(2):
                ps = pst[cls]
                # --- eviction ---
                nev = (ncols - 2) // 2   # 511 full, 161 last
                nc.scalar.copy(evs[cls][:, v0:v0 + nev], ps[:, 2:ncols:2])
                if Wt == 0:
                    nc.scalar.copy(ev0[:, cls:cls + 1], ps[:, 0:1])
                    nc.vector.tensor_tensor(out=tb[cls][:, 0:1],
                                            in0=ev0[:, cls:cls + 1],
                                            in1=ps[:, 1:2], op=MAX)
                    nc.vector.tensor_tensor(
                        out=tb[cls][:, 1:512], in0=evs[cls][:, 0:511],
                        in1=ps[:, 3:1024:2], op=MAX)
                    vend = 512
                else:
                    nt = 512 if full else 161
                    nc.vector.tensor_tensor(
                        out=tb[cls][:, v0:v0 + nt],
                        in0=evs[cls][:, v0 - 1:v0 - 1 + nt],
                        in1=ps[:, 1:2 * nt:2], op=MAX)
                    vend = v0 + nt
                # --- rows newly complete after this tile ---
                mup = min(BROWS, max(0, (vend - 321) // MPITCH + 1))
                if mup - mdone[cls] >= 4 or mup == BROWS:
                    emit_m(cls, mup)
                pup = min(NPOOL, max(0, (mdone[cls] - 1) // 2))
                if pup - pdone[cls] >= 4 or (pup == NPOOL and pup > pdone[cls]):
                    emit_pool(cls, pup)
                    for tgt in (3, 6, 9):
                        if pdone[cls] >= tgt and odone[cls] < tgt:
                            emit_out(cls, tgt)
        for cls in range(2):
            emit_m(cls, BROWS)
            emit_pool(cls, NPOOL)
            emit_out(cls, NPOOL)

    nc.finalize()
    return nc


_NC_CACHE = None


def _get_nc():
    global _NC_CACHE
    if _NC_CACHE is None:
        _NC_CACHE = _build_bass()
    return _NC_CACHE


def build_in_maps(update_location, feature_map, weight, gamma, beta,
                  running_mean, running_var):
    fm = np.asarray(feature_map, np.float32)
    loc = np.asarray(update_location).astype(np.int64)
    wt_ = np.asarray(weight, np.float32)
    gam = np.asarray(gamma, np.float32)
    bet = np.asarray(beta, np.float32)
    mu = np.asarray(running_mean, np.float32)
    var = np.asarray(running_var, np.float32)

    inv = gam / np.sqrt(var + BN_EPS)
    wf = wt_ * inv[None, None, None, :]          # [7,7,3,64]
    bias = bet - mu * inv                        # [64]

    # fm_pad with extra bottom rows so group1 of core 7 stays in bounds
    fmp = np.zeros((H + 2 * PAD + 2, W + 2 * PAD, CIN), np.float32)
    fmp[PAD:PAD + H, PAD:PAD + W] = fm

    # inactive flag per output pixel; cols >= 640 and rows >= 640 inactive
    flag = np.ones((H + 2, W + 6), np.float32)
    flag[loc[:, 0], loc[:, 1]] = 0.0
    flag[:, W:] = 1.0
    flag[H:, :] = 1.0

    # weight rows [128, 192]: pass p block = W'[i, j'+3p, ch, :]
    wrows = np.zeros((64, 192), np.float32)
    for jp in range(3):
        for i in range(KK):
            for ch in range(CIN):
                row = jp * 21 + i * 3 + ch
                for p in range(3):
                    j = jp + 3 * p
                    if j <= 6:
                        wrows[row, 64 * p:64 * p + 64] = wf[i, j, ch]
    wrows[63, 0:64] = NEG
    wt128 = np.concatenate([wrows, wrows], axis=0).astype(ml_dtypes.bfloat16)
    bias128 = np.concatenate([bias, bias]).reshape(128, 1).astype(np.float32)

    in_maps = []
    for k in range(NCORES):
        tblk = np.zeros((128, GPAD), ml_dtypes.bfloat16)
        for g in range(2):
            r0 = 80 * k + 40 * g
            S = np.zeros((64, GROWS, WP), np.float32)
            for jp in range(3):
                for i in range(KK):
                    for ch in range(CIN):
                        S[jp * 21 + i * 3 + ch, :, 0:W + 2 * PAD - jp] = \
                            fmp[r0 + i:r0 + i + GROWS, jp:, ch]
            S[63, :, 0:W + 6] = flag[r0:r0 + GROWS, :]
            tblk[64 * g:64 * g + 64, 0:GCOLS] = \
                S.reshape(64, GCOLS).astype(ml_dtypes.bfloat16)
        in_maps.append({"tbl": tblk, "wt": wt128, "bias": bias128})
    return in_maps


def kernel(update_location, feature_map, weight, gamma, beta, running_mean,
           running_var):
    from concourse.bass_utils import run_bass_kernel_spmd

    in_maps = build_in_maps(update_location, feature_map, weight, gamma, beta,
                            running_mean, running_var)
    nc = _get_nc()
    res = run_bass_kernel_spmd(nc, in_maps, core_ids=list(range(NCORES)))
    # per-core out [128, 6400] bf16: [part, cls*3200 + p*320 + q];
    # partitions 0-63 = channels of the lower sub-band, 64-127 = upper.
    out = np.zeros((NCORES * 40, 320, COUT), np.float32)
    for k in range(NCORES):
        o = np.asarray(res.results[k]["out"], dtype=np.float32)
        o = o.reshape(2, 64, 2, NPOOL, 320)      # [half, ch, cls, p, q]
        for cls in range(2):
            for half in range(2):
                band = 2 * cls + half
                r = 40 * k + 10 * band
                out[r:r + NPOOL] = o[half, :, cls].transpose(1, 2, 0)
    return np.ascontiguousarray(out[:319, :319, :]).astype(np.float32)
